# revision 16
# baseline (speedup 1.0000x reference)
"""Trainium2 Bass kernel for multi-head causal attention with rotary embeddings.

Problem shapes (hardcoded):
  hidden_states [2, 2048, 1024] f32, W_qkv [1024, 3072], W_out [1024, 1024],
  b_out [1024], is_causal scalar. 16 heads x 64 dim, rope theta 10000.

Sharding over 8 cores: core c -> batch c//4, heads 4*(c%4) .. 4*(c%4)+3
(data parallel over batch x tensor parallel over heads; W_qkv column-parallel,
W_out row-parallel; per-core partial outputs are summed on host).

Rope trick: head-dim columns of Wq/Wk are de-interleaved on the host
(pairs (2i, 2i+1) -> (i, i+32)) so on-chip rope is a contiguous half-swap;
scores are invariant because q and k share the permutation.
"""

import numpy as np

B, N, D = 2, 2048, 1024
H, DH = 16, 64
THETA = 10000.0
NCORES = 8
KC = D // 128       # 8 contraction chunks for the projections
NKC = N // 128      # 16 key chunks
NQB = N // 512      # 4 query blocks
NRC = N // 128      # 16 row chunks for the output projection

_compiled = {}


def _build_nc(causal: bool):
    import concourse.bass as bass
    import concourse.tile as tile
    from concourse import bacc, mybir

    f32 = mybir.dt.float32
    Exp = mybir.ActivationFunctionType.Exp
    Copy = mybir.ActivationFunctionType.Copy

    nc = bacc.Bacc("TRN2", target_bir_lowering=False)
    hT_d = nc.dram_tensor("hidden_T", [D, N], f32, kind="ExternalInput")
    w_d = nc.dram_tensor("w_all", [D, 768], f32, kind="ExternalInput")
    wout_d = nc.dram_tensor("w_out", [256, 1024], f32, kind="ExternalInput")
    cos_d = nc.dram_tensor("cos_t", [128, N], f32, kind="ExternalInput")
    sin_d = nc.dram_tensor("sin_t", [128, N], f32, kind="ExternalInput")
    tri_d = nc.dram_tensor("tri", [128, 128], f32, kind="ExternalInput")
    out_d = nc.dram_tensor("out_partial", [N, 1024], f32, kind="ExternalOutput")

    with tile.TileContext(nc) as tc:
        with (
            tc.tile_pool(name="consts", bufs=1) as consts,
            tc.tile_pool(name="qk", bufs=1) as qkp,
            tc.tile_pool(name="vaugp", bufs=1) as vaugp,
            tc.tile_pool(name="atp", bufs=1) as atp,
            tc.tile_pool(name="psA", bufs=2, space="PSUM") as psA,
            tc.tile_pool(name="psB", bufs=4, space="PSUM") as psB,
        ):
            # ---- constants ----
            w_sb = consts.tile([128, KC, 768], f32, tag="w_sb", name="w_sb")
            nc.sync.dma_start(out=w_sb, in_=w_d.rearrange("(a p) c -> p a c", p=128))
            wout_sb = consts.tile([128, 2, 1024], f32, tag="wout_sb", name="wout_sb")
            nc.sync.dma_start(out=wout_sb, in_=wout_d.rearrange("(a p) c -> p a c", p=128))
            cos_sb = consts.tile([128, N], f32, tag="cos_sb", name="cos_sb")
            nc.sync.dma_start(out=cos_sb, in_=cos_d[:, :])
            sin_sb = consts.tile([128, N], f32, tag="sin_sb", name="sin_sb")
            nc.sync.dma_start(out=sin_sb, in_=sin_d[:, :])
            tri_sb = consts.tile([128, 128], f32, tag="tri_sb", name="tri_sb")
            nc.sync.dma_start(out=tri_sb, in_=tri_d[:, :])

            # long-lived activations
            qkT = {}
            for pair in range(2):
                for qk in range(2):
                    t = qkp.tile([128, N], f32, tag=f"qkT{pair}{qk}", name=f"qkT{pair}{qk}")
                    qkT[(pair, qk)] = t
            vaug = {}
            for hcl in range(4):
                t = vaugp.tile([128, NKC, 65], f32, tag=f"vaug{hcl}", name=f"vaug{hcl}")
                nc.vector.memset(t[:, :, 64:65], 1.0)
                vaug[hcl] = t
            A_T = {}
            for pair in range(2):
                A_T[pair] = atp.tile([128, N], f32, tag=f"AT{pair}", name=f"AT{pair}")

            # ---- phase 1: projections (hidden_T staged in a scoped pool) ----
            with tc.tile_pool(name="hT", bufs=KC) as htp:
                hts = []
                for kc in range(KC):
                    ht = htp.tile([128, N], f32, tag="ht", name=f"ht{kc}")
                    nc.sync.dma_start(out=ht, in_=hT_d[kc * 128:(kc + 1) * 128, :])
                    hts.append(ht)

                # fp32 matmuls only support a single sync-wait command, so each
                # matmul may depend on at most one not-yet-observed semaphore.
                # This tiny warm-up matmul absorbs the w_sb DMA edge so the
                # first real matmul only waits on its ht DMA.
                warm = psA.tile([1, 1], f32, tag="psA", name="warm")
                nc.tensor.matmul(warm, lhsT=w_sb[:, 0, 0:1], rhs=w_sb[:, 0, 0:1],
                                 start=True, stop=True)

                # q,k projections: weights stationary, output transposed.
                # psum tile [128, 1024] covers two 512-row blocks (2 banks).
                for pair in range(2):
                    for qk in range(2):
                        col0 = (pair * 2 + qk) * 128
                        dst = qkT[(pair, qk)]
                        scale = 0.125 if qk == 0 else 1.0
                        for rbp in range(2):
                            qk_ps = psA.tile([128, 1024], f32, tag="psA", name="qk_ps")
                            for half in range(2):
                                rb = rbp * 2 + half
                                for kc in range(KC):
                                    nc.tensor.matmul(
                                        qk_ps[:, half * 512:(half + 1) * 512],
                                        lhsT=w_sb[:, kc, col0:col0 + 128],
                                        rhs=hts[kc][:, rb * 512:(rb + 1) * 512],
                                        start=(kc == 0), stop=(kc == KC - 1),
                                    )
                            nc.scalar.activation(
                                dst[:, rbp * 1024:(rbp + 1) * 1024],
                                qk_ps[:, :], func=Copy, scale=scale,
                            )

                # v projection: hidden stationary, output natural [rows, 4h*64].
                for rc in range(NRC):
                    v_ps = psB.tile([128, 256], f32, tag="psB", name="v_ps")
                    for kc in range(KC):
                        nc.tensor.matmul(
                            v_ps,
                            lhsT=hts[kc][:, rc * 128:(rc + 1) * 128],
                            rhs=w_sb[:, kc, 512:768],
                            start=(kc == 0), stop=(kc == KC - 1),
                        )
                    for hcl in range(4):
                        nc.scalar.copy(
                            vaug[hcl][:, rc, 0:64],
                            v_ps[:, hcl * 64:(hcl + 1) * 64],
                        )

                # rope on q,k (in sbuf, transposed layout)
                with tc.tile_pool(name="ropep", bufs=1) as ropep:
                    for pair in range(2):
                        for qk in range(2):
                            t = qkT[(pair, qk)]
                            tmp = ropep.tile([128, N], f32, tag="ropetmp", name="ropetmp")
                            # sin table is indexed by SOURCE rows (walrus requires
                            # both SBUF inputs of tensor_tensor to share base partition)
                            for h2 in range(2):
                                b0 = h2 * 64
                                nc.vector.tensor_mul(
                                    tmp[b0:b0 + 32, :], t[b0 + 32:b0 + 64, :], sin_sb[b0 + 32:b0 + 64, :])
                                nc.vector.tensor_mul(
                                    tmp[b0 + 32:b0 + 64, :], t[b0:b0 + 32, :], sin_sb[b0:b0 + 32, :])
                            nc.vector.tensor_mul(t, t, cos_sb)
                            nc.vector.tensor_add(t, t, tmp)

            # ---- phase 2+3: attention and output projection ----
            with (
                tc.tile_pool(name="psbp", bufs=3) as psbp,
                tc.tile_pool(name="smallp", bufs=2) as smallp,
                tc.tile_pool(name="outp", bufs=2) as outp,
            ):
                # warm-up: absorb the ACT edge (v_aug copies / qk evac) before
                # the first score matmul, which already waits on the DVE rope.
                warm2 = psA.tile([1, 1], f32, tag="psA", name="warm2")
                nc.tensor.matmul(warm2, lhsT=vaug[0][:, 0, 0:1], rhs=vaug[0][:, 0, 0:1],
                                 start=True, stop=True)
                for pair in range(2):
                    qT = qkT[(pair, 0)]
                    kT = qkT[(pair, 1)]
                    for qb in range(NQB):
                        kmax = 4 * qb + 3 if causal else NKC - 1
                        pvs = []
                        for h2 in range(2):
                            pv = psB.tile([65, 512], f32, tag="psB", name=f"pv{h2}")
                            pvs.append(pv)
                        pending = None  # software-pipeline PV one step behind scores
                        for kc in range(kmax + 1):
                            qlo = max(0, kc * 128 - qb * 512) if causal else 0
                            st = psA.tile([128, 1024], f32, tag="psA", name="st")
                            psb = psbp.tile([128, 1024], f32, tag="psb", name="psb")
                            for h2 in range(2):
                                b0 = h2 * 64
                                nc.tensor.matmul(
                                    st[:, h2 * 512 + qlo:(h2 + 1) * 512],
                                    lhsT=kT[b0:b0 + 64, kc * 128:(kc + 1) * 128],
                                    rhs=qT[b0:b0 + 64, qb * 512 + qlo:(qb + 1) * 512],
                                    start=True, stop=True,
                                )
                            if causal and kc >= 4 * qb:
                                # additive causal mask (0 / -1e9) applied to the
                                # diagonal block in psum BEFORE exp, so psb has a
                                # single writer (ACT) and PV needs only one wait.
                                for h2 in range(2):
                                    nc.vector.tensor_add(
                                        st[:, h2 * 512 + qlo:h2 * 512 + qlo + 128],
                                        st[:, h2 * 512 + qlo:h2 * 512 + qlo + 128],
                                        tri_sb)
                            if qlo == 0:
                                nc.scalar.activation(psb[:, :], st[:, :], func=Exp)
                            else:
                                for h2 in range(2):
                                    nc.scalar.activation(
                                        psb[:, h2 * 512 + qlo:(h2 + 1) * 512],
                                        st[:, h2 * 512 + qlo:(h2 + 1) * 512], func=Exp)
                            if pending is not None:
                                pkc, pqlo, ppsb = pending
                                for h2 in range(2):
                                    nc.tensor.matmul(
                                        pvs[h2][:, pqlo:],
                                        lhsT=vaug[pair * 2 + h2][:, pkc, :],
                                        rhs=ppsb[:, h2 * 512 + pqlo:(h2 + 1) * 512],
                                        start=(pkc == 0), stop=False,
                                    )
                            pending = (kc, qlo, psb)
                        pkc, pqlo, ppsb = pending
                        for h2 in range(2):
                            nc.tensor.matmul(
                                pvs[h2][:, pqlo:],
                                lhsT=vaug[pair * 2 + h2][:, pkc, :],
                                rhs=ppsb[:, h2 * 512 + pqlo:(h2 + 1) * 512],
                                start=(pkc == 0), stop=True,
                            )
                        for h2 in range(2):
                            # evacuate via ACT so the psum slot's last consumer
                            # stays on the ACT semaphore (fp32 1-wait rule)
                            pvc = smallp.tile([65, 512], f32, tag="pvc", name="pvc")
                            nc.scalar.copy(pvc, pvs[h2])
                            recip = smallp.tile([1, 512], f32, tag="recip", name="recip")
                            nc.vector.reciprocal(recip, pvc[64:65, :])
                            bc = smallp.tile([64, 512], f32, tag="bc", name="bc")
                            nc.gpsimd.partition_broadcast(bc, recip)
                            nc.vector.tensor_mul(
                                A_T[pair][h2 * 64:(h2 + 1) * 64, qb * 512:(qb + 1) * 512],
                                pvc[0:64, :], bc)

                # ---- output projection (row-parallel partial) ----
                # warm-up: absorb the wout_sb DMA edge
                warm3 = psA.tile([1, 1], f32, tag="psA", name="warm3")
                nc.tensor.matmul(warm3, lhsT=wout_sb[:, 0, 0:1], rhs=wout_sb[:, 0, 0:1],
                                 start=True, stop=True)
                for rc in range(NRC):
                    op_ps = psA.tile([128, 1024], f32, tag="psA", name="op_ps")
                    for half in range(2):
                        for pair in range(2):
                            nc.tensor.matmul(
                                op_ps[:, half * 512:(half + 1) * 512],
                                lhsT=A_T[pair][:, rc * 128:(rc + 1) * 128],
                                rhs=wout_sb[:, pair, half * 512:(half + 1) * 512],
                                start=(pair == 0), stop=(pair == 1),
                            )
                    o_sb = outp.tile([128, 1024], f32, tag="o_sb", name="o_sb")
                    nc.scalar.copy(o_sb[:, 0:512], op_ps[:, 0:512])
                    nc.scalar.copy(o_sb[:, 512:1024], op_ps[:, 512:1024])
                    nc.sync.dma_start(
                        out=out_d[rc * 128:(rc + 1) * 128, :], in_=o_sb)

    nc.compile()
    return nc


def _host_inputs(hidden_states, W_qkv, W_out):
    """Build the 8 per-core input maps."""
    hidden = np.ascontiguousarray(hidden_states, dtype=np.float32)
    W_qkv = np.asarray(W_qkv, dtype=np.float32)
    W_out = np.asarray(W_out, dtype=np.float32)
    Wq, Wk, Wv = W_qkv[:, :1024], W_qkv[:, 1024:2048], W_qkv[:, 2048:]

    perm = np.concatenate([np.arange(0, 64, 2), np.arange(1, 64, 2)])

    invf = THETA ** (-np.arange(0, 32, dtype=np.float64) * 2.0 / 64.0)
    ang = np.arange(N, dtype=np.float64)[:, None] * invf[None, :]  # [N, 32]
    jj = np.arange(64)
    cos64 = np.cos(ang)[:, jj % 32].T
    sin64 = np.sin(ang)[:, jj % 32].T
    # row r holds the sin factor applied when row r is the SOURCE of the
    # half-swap: rows j<32 feed dst j+32 with +sin; rows j>=32 feed dst j-32
    # with -sin.
    sgn = np.where(jj < 32, 1.0, -1.0)[:, None]
    cos_t = np.ascontiguousarray(np.tile(cos64, (2, 1)), dtype=np.float32)
    sin_t = np.ascontiguousarray(np.tile(sgn * sin64, (2, 1)), dtype=np.float32)
    # additive mask: 0 where q >= k (valid), -1e9 where masked
    tri = np.ascontiguousarray(
        np.where(np.arange(128)[None, :] >= np.arange(128)[:, None], 0.0, -1e9),
        dtype=np.float32)

    hT = [np.ascontiguousarray(hidden[b].T) for b in range(B)]

    in_maps = []
    for c in range(NCORES):
        bb = c // 4
        bh = (c % 4) * 4

        def qk_cols(pair):
            cols = []
            for h2 in range(2):
                hh = bh + pair * 2 + h2
                cols.extend(hh * 64 + perm)
            return np.array(cols)

        w_all = np.ascontiguousarray(np.concatenate(
            [Wq[:, qk_cols(0)], Wk[:, qk_cols(0)],
             Wq[:, qk_cols(1)], Wk[:, qk_cols(1)],
             Wv[:, bh * 64:(bh + 4) * 64]], axis=1), dtype=np.float32)
        wout_c = np.ascontiguousarray(W_out[bh * 64:(bh + 4) * 64, :], dtype=np.float32)
        in_maps.append({
            "hidden_T": hT[bb],
            "w_all": w_all,
            "w_out": wout_c,
            "cos_t": cos_t,
            "sin_t": sin_t,
            "tri": tri,
        })
    return in_maps


def _pjrt_exec(nc, in_maps, time_iters=0):
    """Mirror of bass2jax.run_bass_via_pjrt's multi-core path, with the jitted
    executable kept so repeated timed invocations are possible."""
    import jax
    import jax.numpy as jnp
    from jax.experimental.shard_map import shard_map
    from jax.sharding import Mesh, PartitionSpec
    import concourse.mybir as mybir
    from concourse.bass2jax import (
        _bass_exec_p, install_neuronx_cc_hook, partition_id_tensor)

    install_neuronx_cc_hook()
    n_cores = len(in_maps)
    partition_name = nc.partition_id_tensor.name if nc.partition_id_tensor else None
    in_names, out_names, out_avals = [], [], []
    for alloc in nc.m.functions[0].allocations:
        if not isinstance(alloc, mybir.MemoryLocationSet):
            continue
        name = alloc.memorylocations[0].name
        if alloc.kind == "ExternalInput":
            if name != partition_name:
                in_names.append(name)
        elif alloc.kind == "ExternalOutput":
            out_names.append(name)
            out_avals.append(
                jax.core.ShapedArray(tuple(alloc.tensor_shape), mybir.dt.np(alloc.dtype)))
    n_params = len(in_names)
    all_in_names = list(in_names) + list(out_names)
    if partition_name is not None:
        all_in_names.append(partition_name)

    def _body(*args):
        operands = list(args)
        if partition_name is not None:
            operands.append(partition_id_tensor())
        outs = _bass_exec_p.bind(
            *operands,
            out_avals=tuple(out_avals),
            in_names=tuple(all_in_names),
            out_names=tuple(out_names),
            lowering_input_output_aliases=(),
            sim_require_finite=True,
            sim_require_nnan=True,
            nc=nc,
        )
        return tuple(outs)

    devices = jax.devices()[:n_cores]
    mesh = Mesh(np.asarray(devices), ("core",))
    n_outs = len(out_names)
    _inner = shard_map(
        _body, mesh=mesh,
        in_specs=(PartitionSpec("core"),) * (n_params + n_outs),
        out_specs=(PartitionSpec("core"),) * n_outs,
        check_rep=False)
    donate = tuple(range(n_params, n_params + n_outs))
    fn = jax.jit(_inner, donate_argnums=donate, keep_unused=True)

    concat_in = [
        np.concatenate([np.asarray(in_maps[c][name]) for c in range(n_cores)], axis=0)
        for name in in_names
    ]
    from jax.sharding import NamedSharding
    sharding = NamedSharding(mesh, PartitionSpec("core"))
    concat_dev = [jax.device_put(a, sharding) for a in concat_in]

    def _zero_set():
        return [
            jax.device_put(
                np.zeros((n_cores * a.shape[0],) + tuple(a.shape[1:]), a.dtype),
                sharding)
            for a in out_avals
        ]

    out_arrs = jax.block_until_ready(fn(*concat_dev, *_zero_set()))

    exec_ns = None
    if time_iters:
        import time as _time
        zero_sets = [_zero_set() for _ in range(time_iters)]
        jax.block_until_ready(zero_sets)
        t0 = _time.perf_counter()
        outs = [fn(*concat_dev, *zero_sets[i]) for i in range(time_iters)]
        jax.block_until_ready(outs)
        t1 = _time.perf_counter()
        exec_ns = (t1 - t0) / time_iters * 1e9

    results = [
        {name: np.asarray(out_arrs[i]).reshape(n_cores, *out_avals[i].shape)[c]
         for i, name in enumerate(out_names)}
        for c in range(n_cores)
    ]
    return results, exec_ns


def run(hidden_states, W_qkv, W_out, b_out, is_causal, time_iters=0):
    causal = bool(int(np.asarray(is_causal)))
    key = ("nc", causal)
    if key not in _compiled:
        _compiled[key] = _build_nc(causal)
    nc = _compiled[key]

    in_maps = _host_inputs(hidden_states, W_qkv, W_out)
    results, exec_ns = _pjrt_exec(nc, in_maps, time_iters=time_iters)

    out = np.zeros((N * B, 1024), dtype=np.float32).reshape(B, N, 1024)
    for c in range(NCORES):
        out[c // 4] += results[c]["out_partial"]
    out += np.asarray(b_out, dtype=np.float32)[None, None, :]
    return out, exec_ns


def kernel(hidden_states, W_qkv, W_out, b_out, is_causal):
    out, _ = run(hidden_states, W_qkv, W_out, b_out, is_causal)
    return out
